# revision 11
# baseline (speedup 1.0000x reference)
"""Trainium2 Bass kernel for the 16-head MHA problem (B=4, S=2048, D=1024).

Sharding: 8 cores = 4 batches x 2 head-groups (8 heads each).
Per core: Q/K/V projections (column-split), attention for 8 heads with the
S^T (keys-on-partitions) orientation, row-split output projection; the two
head-group partial outputs per batch are summed on the host.

The reference adds mask*2^32 to the scores BEFORE the 1/sqrt(dk) scale and
softmax.  In fp32 this collapses every masked score to exactly 2^29 (after
the scale) and drives every unmasked score in such rows to exp(-2^29)=0.
We exploit the block structure of the mask (classified at run time from the
actual mask tensor):
  - all-zero mask blocks: contribute nothing -> skipped entirely
  - all-one blocks: P == 1 exactly -> rank-1 (column-sum of V) updates
  - mixed blocks: S matmul + fp32 mask add (reproduces the collapse) + exp
Rows with no masked entry at all (only the last row for the causal-complement
mask) get a true softmax; those are patched on the host from K/V shipped back
from the device.
"""

import numpy as np
import ml_dtypes

import concourse.bass as bass
import concourse.mybir as mybir
import concourse.tile as tile
from concourse import bacc, bass_utils

# ---------------------------------------------------------------- constants
B, S, D = 4, 2048, 1024
HEADS, DK = 16, 64
HG = 2                      # tensor-parallel head groups
HPG = HEADS // HG           # 8 heads per core
DG = HPG * DK               # 512 projection width per core
N_CORES = B * HG
KT = 128                    # seq tile on the key axis (partitions of S^T)
QC = 512                    # seq column block on the query axis
NKT = S // KT               # 16
NQC = S // QC               # 4
NDT = DG // 128             # 4 planes of Q^T/K^T/O^T
NKD = D // 128              # 8 k-tiles of the model dim
MASK_CONST = 4294967296.0   # +2^32, faithful to the reference
SCALE = 1.0 / np.sqrt(np.float32(DK))   # 1/8
MASKC = np.float32(2.0 ** 29)           # mask * 2^32 * SCALE
NEG_BIAS = -float(2.0 ** 29)

F32 = mybir.dt.float32
F32R = mybir.dt.float32r
BF16 = mybir.dt.bfloat16


def _round_f32r(a: np.ndarray) -> np.ndarray:
    """Round fp32 to the fp32r grid (13-bit mantissa, RNE) like the HW cast."""
    u = np.ascontiguousarray(a, dtype=np.float32).view(np.uint32)
    r = (u + np.uint32(0x1FF) + ((u >> np.uint32(10)) & np.uint32(1))) & np.uint32(0xFFFFFC00)
    return r.view(np.float32)


# ------------------------------------------------------------ classification
def _classify(mask2d: np.ndarray):
    """Block classes per (qc, kt): 0 = all-zero, 1 = all-one, 2 = mixed.
    Also the list of rows with no exactly-1.0 mask entry (host-patched)."""
    m = mask2d
    has1 = (m == 1.0).any(axis=1)
    qfix = np.where(~has1)[0]
    cls = np.empty((NQC, NKT), dtype=np.int8)
    for c in range(NQC):
        sub = m[c * QC:(c + 1) * QC]
        for t in range(NKT):
            blk = sub[:, t * KT:(t + 1) * KT]
            if (blk == 0.0).all():
                cls[c, t] = 0
            elif (blk == 1.0).all():
                cls[c, t] = 1
            else:
                cls[c, t] = 2
    return cls, qfix


# ------------------------------------------------------------- kernel build
def _build(cls: np.ndarray, need_dump: bool):
    comp = [[t for t in range(NKT) if cls[c, t] == 2] for c in range(NQC)]
    ones = [[t for t in range(NKT) if cls[c, t] == 1] for c in range(NQC)]
    n_comp = [len(x) for x in comp]
    n_comp_total = max(1, sum(n_comp))
    comp_off = np.cumsum([0] + n_comp)  # offset of qc's blocks in maskct

    nc = bacc.Bacc("TRN2", target_bir_lowering=False, debug=False,
                   num_devices=N_CORES)

    dt_in = {}
    def din(name, shape, dt):
        dt_in[name] = nc.dram_tensor(name, shape, dt, kind="ExternalInput").ap()
        return dt_in[name]

    xq = din("xq", (128, NKD, S), F32R)        # x_q^T  [p, kt, s]
    xk = din("xk", (128, NKD, S), F32R)
    xv = din("xv", (128, NKD, S), F32R)
    wq = din("wq", (128, NKD, DG), F32R)       # [p, kt, dout]
    wk = din("wk", (128, NKD, DG), F32R)       # pre-scaled by 1/8
    wv = din("wv", (128, NKD, DG), F32R)
    wo = din("wo", (128, NDT, D), F32R)        # [p, plane, dmodel]
    bq = din("bq", (128, NDT), F32)
    bk = din("bk", (128, NDT), F32)            # pre-scaled by 1/8
    bv = din("bv", (1, DG), F32R)
    bo = din("bo", (1, D), F32R)
    maskct = din("maskct", (128, n_comp_total, QC), BF16)  # 2^29 * mask^T blocks

    out = nc.dram_tensor("out", (S, D), F32, kind="ExternalOutput").ap()
    if need_dump:
        kt_out = nc.dram_tensor("kt_out", (128, NDT, S), F32R,
                                kind="ExternalOutput").ap()
        v_out = nc.dram_tensor("v_out", (128, NKT, HPG, DK), F32R,
                               kind="ExternalOutput").ap()

    with tile.TileContext(nc) as tc:
        with (
            tc.tile_pool(name="res", bufs=1) as res,
            tc.tile_pool(name="small", bufs=1) as small,
        ):
            # ---------------- resident tensors
            qt_sb = res.tile([128, NDT, S], F32R, tag="qt")     # Q^T
            kt_sb = res.tile([128, NDT, S], F32R, tag="kt")     # K^T (scaled)
            v_sb = res.tile([128, NKT, HPG, DK + 1], F32R, tag="v")
            wo_sb = res.tile([128, NDT, D], F32R, tag="wo")
            bq_sb = small.tile([128, NDT], F32, tag="bq")
            bk_sb = small.tile([128, NDT], F32, tag="bk")
            bv_sb = small.tile([1, DG], F32R, tag="bv")
            bo_sb = small.tile([1, D], F32R, tag="bo")
            ones_col = small.tile([1, 128], F32R, tag="onesc")
            ones_row = small.tile([1, QC], F32R, tag="onesr")
            ones_k = small.tile([128, 1], F32R, tag="onesk")
            negb = small.tile([128, 1], F32, tag="negb")
            sfx_sb = small.tile([1, NQC, HPG, DK + 1], F32R, tag="sfx")

            nc.vector.memset(ones_col[:].bitcast(F32), 1.0)
            nc.vector.memset(ones_row[:].bitcast(F32), 1.0)
            nc.vector.memset(ones_k[:].bitcast(F32), 1.0)
            nc.vector.memset(negb[:], NEG_BIAS)
            nc.vector.memset(v_sb[:, :, :, DK:DK + 1].bitcast(F32), 1.0)
            nc.sync.dma_start(wo_sb[:], wo[:])
            nc.sync.dma_start(bq_sb[:], bq[:])
            nc.sync.dma_start(bk_sb[:], bk[:])
            nc.sync.dma_start(bv_sb[:], bv[:])
            nc.sync.dma_start(bo_sb[:], bo[:])

            with (
                tc.tile_pool(name="xstage", bufs=12) as xstage,
                tc.tile_pool(name="wstage", bufs=2) as wstage,
                tc.tile_pool(name="ppsum", bufs=4, space="PSUM") as ppsum,
                tc.tile_pool(name="cpsum", bufs=2, space="PSUM") as cpsum,
            ):
                # ---------------- K^T and Q^T projections
                for which, xin, w_in, b_sb, dst in (
                    ("k", xk, wk, bk_sb, kt_sb),
                    ("q", xq, wq, bq_sb, qt_sb),
                ):
                    wtile = wstage.tile([128, NKD, DG], F32R, tag="w")
                    nc.sync.dma_start(wtile[:], w_in[:])
                    for c in range(NQC):
                        xt = []
                        for k in range(NKD):
                            t = xstage.tile([128, QC], F32R, tag="xt")
                            nc.sync.dma_start(t[:], xin[:, k, c * QC:(c + 1) * QC])
                            xt.append(t)
                        for dtl in range(NDT):
                            ps = ppsum.tile([128, QC], F32, tag="pp")
                            for k in range(NKD):
                                nc.tensor.matmul(
                                    ps[:], wtile[:, k, dtl * 128:(dtl + 1) * 128],
                                    xt[k][:], start=(k == 0), stop=(k == NKD - 1))
                            nc.scalar.activation(
                                dst[:, dtl, c * QC:(c + 1) * QC], ps[:],
                                mybir.ActivationFunctionType.Identity,
                                bias=b_sb[:, dtl:dtl + 1])
                if need_dump:
                    nc.sync.dma_start(kt_out[:], kt_sb[:])

                # ---------------- V projection (natural orientation)
                wvt = wstage.tile([128, NKD, DG], F32R, tag="w")
                nc.sync.dma_start(wvt[:], wv[:])
                for g in range(4):
                    xt = []
                    for k in range(NKD):
                        t = xstage.tile([128, QC], F32R, tag="xt")
                        nc.sync.dma_start(t[:], xv[:, k, g * QC:(g + 1) * QC])
                        xt.append(t)
                    for sl in range(4):
                        st = g * 4 + sl
                        ps = ppsum.tile([128, DG], F32, tag="pp")
                        for k in range(NKD):
                            nc.tensor.matmul(
                                ps[:], xt[k][:, sl * 128:(sl + 1) * 128],
                                wvt[:, k, :], start=(k == 0), stop=False)
                        nc.tensor.matmul(ps[:], ones_col[:, :], bv_sb[:],
                                         start=False, stop=True)
                        for h in range(HPG):
                            nc.vector.tensor_copy(
                                v_sb[:, st, h, 0:DK], ps[:, h * DK:(h + 1) * DK])
                if need_dump:
                    nc.sync.dma_start(v_out[:], v_sb[:, :, :, 0:DK])

                # ------------- column sums of V_ext over each ONES tile set
                for c in range(NQC):
                    if not ones[c]:
                        continue
                    for hh in range(2):
                        cp = cpsum.tile([1, 4, DK + 1], F32, tag="cp")
                        for i, t in enumerate(ones[c]):
                            nc.tensor.matmul(
                                cp[:], ones_k[:],
                                v_sb[:, t, hh * 4:(hh + 1) * 4, :],
                                start=(i == 0), stop=(i == len(ones[c]) - 1))
                        nc.vector.tensor_copy(
                            sfx_sb[:, c, hh * 4:(hh + 1) * 4, :], cp[:])

            # ---------------- attention + output projection
            with (
                tc.tile_pool(name="mstage", bufs=2) as mstage,
                tc.tile_pool(name="ptpool", bufs=2) as ptpool,
                tc.tile_pool(name="otpool", bufs=2) as otpool,
                tc.tile_pool(name="osb", bufs=2) as osb,
                tc.tile_pool(name="outsb", bufs=2) as outsb,
                tc.tile_pool(name="rcpool", bufs=2) as rcpool,
                tc.tile_pool(name="spsum", bufs=2, space="PSUM") as spsum,
                tc.tile_pool(name="opsum", bufs=2, space="PSUM") as opsum,
                tc.tile_pool(name="bpsum", bufs=1, space="PSUM") as bpsum,
                tc.tile_pool(name="wpsum", bufs=1, space="PSUM") as wpsum,
            ):
                for c in range(NQC):
                    nct = n_comp[c]
                    ot_qc = otpool.tile([128, NDT, QC], F32R, tag="ot")
                    mtile = None
                    if nct:
                        mtile = mstage.tile([128, nct, QC], BF16, tag=f"m{nct}")
                        nc.sync.dma_start(
                            mtile[:], maskct[:, comp_off[c]:comp_off[c] + nct, :])
                    for h in range(HPG):
                        p0 = 64 * (h % 2)
                        pl = h // 2
                        if nct == 0 and not ones[c]:
                            continue
                        o_ps = opsum.tile([DK + 1, QC], F32, tag="op")
                        first = True
                        if nct:
                            pt = ptpool.tile([128, nct, QC], F32R, tag=f"p{nct}")
                            npair = (nct + 1) // 2
                            for j2 in range(npair):
                                j0 = 2 * j2
                                w = min(2, nct - j0)
                                sp = spsum.tile([128, 2, QC], F32, tag="sp")
                                for jj in range(w):
                                    t = comp[c][j0 + jj]
                                    nc.tensor.matmul(
                                        sp[:, jj, :],
                                        kt_sb[p0:p0 + 64, pl, t * KT:(t + 1) * KT],
                                        qt_sb[p0:p0 + 64, pl, c * QC:(c + 1) * QC],
                                        start=True, stop=True)
                                nc.vector.tensor_tensor(
                                    sp[:, 0:w, :], sp[:, 0:w, :],
                                    mtile[:, j0:j0 + w, :], mybir.AluOpType.add)
                                nc.scalar.activation(
                                    pt[:, j0:j0 + w, :], sp[:, 0:w, :],
                                    mybir.ActivationFunctionType.Exp,
                                    bias=negb[:])
                            for j in range(nct):
                                t = comp[c][j]
                                nc.tensor.matmul(
                                    o_ps[:], v_sb[:, t, h, :], pt[:, j, :],
                                    start=first, stop=False)
                                first = False
                        if ones[c]:
                            nc.tensor.matmul(o_ps[:], sfx_sb[:, c, h, :],
                                             ones_row[:], start=first, stop=True)
                        # normalize: divide by the V_ext ones-column sums
                        rc = rcpool.tile([1, QC], F32, tag="rc")
                        rcr = rcpool.tile([1, QC], F32R, tag="rcr")
                        nc.vector.reciprocal(rc[:], o_ps[DK:DK + 1, :])
                        nc.vector.tensor_copy(rcr[:], rc[:])
                        b_ps = bpsum.tile([64, QC], F32, tag="bp")
                        nc.tensor.matmul(b_ps[:], ones_col[:, 0:64], rcr[:],
                                         start=True, stop=True)
                        o_sb = osb.tile([64, QC], F32, tag="ob")
                        nc.scalar.copy(o_sb[:], o_ps[0:DK, :])
                        nc.vector.tensor_mul(ot_qc[p0:p0 + 64, pl, :],
                                             o_sb[:], b_ps[:])
                    # ---------- output projection for this q block
                    for qt in range(QC // 128):
                        for ncol in range(D // QC):
                            wp = wpsum.tile([128, QC], F32, tag="wp")
                            for pl in range(NDT):
                                nc.tensor.matmul(
                                    wp[:], ot_qc[:, pl, qt * 128:(qt + 1) * 128],
                                    wo_sb[:, pl, ncol * QC:(ncol + 1) * QC],
                                    start=(pl == 0), stop=False)
                            nc.tensor.matmul(wp[:], ones_col[:, :],
                                             bo_sb[:, ncol * QC:(ncol + 1) * QC],
                                             start=False, stop=True)
                            ob = outsb.tile([128, QC], F32, tag="outb")
                            nc.scalar.copy(ob[:], wp[:])
                            nc.sync.dma_start(
                                out[c * QC + qt * 128:c * QC + (qt + 1) * 128,
                                    ncol * QC:(ncol + 1) * QC], ob[:])

    nc.compile()
    return nc


# ------------------------------------------------------------- host wrapper
_CACHE: dict = {}


def _get_kernel(cls_key, cls, need_dump):
    if cls_key not in _CACHE:
        _CACHE[cls_key] = _build(cls, need_dump)
    return _CACHE[cls_key]


def kernel(queries, keys, values, mask, Wq, bq, Wk, bk, Wv, bv, Wo, bo):
    queries = np.asarray(queries, dtype=np.float32)
    keys = np.asarray(keys, dtype=np.float32)
    values = np.asarray(values, dtype=np.float32)
    mask2d = np.ascontiguousarray(np.asarray(mask, dtype=np.float32).reshape(S, S))
    Wq = np.asarray(Wq, dtype=np.float32); bq_ = np.asarray(bq, dtype=np.float32)
    Wk = np.asarray(Wk, dtype=np.float32); bk_ = np.asarray(bk, dtype=np.float32)
    Wv = np.asarray(Wv, dtype=np.float32); bv_ = np.asarray(bv, dtype=np.float32)
    Wo = np.asarray(Wo, dtype=np.float32); bo_ = np.asarray(bo, dtype=np.float32)

    cls, qfix = _classify(mask2d)
    need_dump = len(qfix) > 0
    comp = [[t for t in range(NKT) if cls[c, t] == 2] for c in range(NQC)]
    n_comp_total = max(1, sum(len(x) for x in comp))
    cls_key = (cls.tobytes(), need_dump)
    nc = _get_kernel(cls_key, cls, need_dump)

    # pack the mixed mask blocks: [p, j, col] = 2^29 * mask[q, k]
    maskct = np.zeros((128, n_comp_total, QC), dtype=np.float32)
    j = 0
    for c in range(NQC):
        for t in comp[c]:
            blk = mask2d[c * QC:(c + 1) * QC, t * KT:(t + 1) * KT]  # [q, k]
            maskct[:, j, :] = blk.T * MASKC
            j += 1
    maskct = maskct.astype(ml_dtypes.bfloat16)

    def prep_xt(x):  # (S, D) -> [128, NKD, S] transposed fp32r
        xt = _round_f32r(x.T)                     # [D, S]
        return np.ascontiguousarray(xt.reshape(NKD, 128, S).transpose(1, 0, 2))

    xts = {}
    in_maps = []
    for core in range(N_CORES):
        b, hg = divmod(core, HG)
        if b not in xts:
            xts[b] = (prep_xt(queries[b]), prep_xt(keys[b]), prep_xt(values[b]))
        sl = slice(hg * DG, (hg + 1) * DG)
        im = {
            "xq": xts[b][0], "xk": xts[b][1], "xv": xts[b][2],
            "wq": np.ascontiguousarray(
                _round_f32r(Wq[:, sl]).reshape(NKD, 128, DG).transpose(1, 0, 2)),
            "wk": np.ascontiguousarray(
                _round_f32r(Wk[:, sl] * np.float32(SCALE)).reshape(NKD, 128, DG).transpose(1, 0, 2)),
            "wv": np.ascontiguousarray(
                _round_f32r(Wv[:, sl]).reshape(NKD, 128, DG).transpose(1, 0, 2)),
            "wo": np.ascontiguousarray(
                _round_f32r(Wo[sl, :]).reshape(NDT, 128, D).transpose(1, 0, 2)),
            "bq": np.ascontiguousarray(bq_[sl].reshape(NDT, 128).T),
            "bk": np.ascontiguousarray((bk_[sl] * np.float32(SCALE)).reshape(NDT, 128).T),
            "bv": _round_f32r(bv_[sl]).reshape(1, DG),
            "bo": (_round_f32r(bo_).reshape(1, D) if hg == 0
                   else np.zeros((1, D), np.float32)),
            "maskct": maskct,
        }
        in_maps.append(im)

    res = bass_utils.run_bass_kernel_spmd(
        nc, in_maps, core_ids=list(range(N_CORES)))

    global LAST_RESULTS, LAST_IN_MAPS
    LAST_RESULTS = res
    LAST_IN_MAPS = in_maps

    out = np.empty((B, S, D), dtype=np.float32)
    for b in range(B):
        out[b] = res.results[b * HG]["out"] + res.results[b * HG + 1]["out"]

    # ---------------- host patch for rows with no masked entry
    if need_dump:
        q = qfix
        mrow = mask2d[q] * np.float32(MASK_CONST)          # [nq, S]
        for b in range(B):
            acc = np.zeros((len(q), D), dtype=np.float32)
            for hg in range(HG):
                r = res.results[b * HG + hg]
                ktd = r["kt_out"]                           # [128, NDT, S]
                K = ktd.transpose(1, 0, 2).reshape(DG, S)   # [dg, S] (scaled)
                vd = r["v_out"]                             # [128, NKT, HPG, DK]
                V = vd.transpose(1, 0, 2, 3).reshape(S, HPG, DK)
                sl = slice(hg * DG, (hg + 1) * DG)
                Qr = queries[b][q] @ Wq[:, sl] + bq_[sl]    # [nq, DG]
                Oc = np.empty((len(q), DG), dtype=np.float32)
                for h in range(HPG):
                    s = (Qr[:, h * DK:(h + 1) * DK] @ K[h * DK:(h + 1) * DK, :])
                    y = s.astype(np.float32) + mrow * np.float32(SCALE)
                    y = y - y.max(axis=1, keepdims=True)
                    e = np.exp(y, dtype=np.float32)
                    p = e / e.sum(axis=1, keepdims=True)
                    Oc[:, h * DK:(h + 1) * DK] = p @ V[:, h, :]
                acc += Oc @ Wo[sl, :]
            out[b][q] = acc + bo_
    return out.reshape(B, S, D)


# revision 13
# speedup vs baseline: 1.0554x; 1.0554x over previous
"""Trainium2 Bass kernel for the 16-head MHA problem (B=4, S=2048, D=1024).

Sharding: 8 cores = 4 batches x 2 head-groups (8 heads each).
Per core: Q/K/V projections (column-split), attention for 8 heads with the
S^T (keys-on-partitions) orientation, row-split output projection; the two
head-group partial outputs per batch are summed on the host.

The reference adds mask*2^32 to the scores BEFORE the 1/sqrt(dk) scale and
softmax.  In fp32 this collapses every masked score to exactly 2^29 (after
the scale) and drives every unmasked score in such rows to exp(-2^29)=0.
We exploit the block structure of the mask (classified at run time from the
actual mask tensor):
  - all-zero mask blocks: contribute nothing -> skipped entirely
  - all-one blocks: P == 1 exactly -> rank-1 (column-sum of V) updates
  - mixed blocks: S matmul + fp32 mask add (reproduces the collapse) + exp
Rows with no masked entry at all (only the last row for the causal-complement
mask) get a true softmax; those are patched on the host from K/V shipped back
from the device.
"""

import numpy as np
import ml_dtypes

import concourse.bass as bass
import concourse.mybir as mybir
import concourse.tile as tile
from concourse import bacc, bass_utils

# ---------------------------------------------------------------- constants
B, S, D = 4, 2048, 1024
HEADS, DK = 16, 64
HG = 2                      # tensor-parallel head groups
HPG = HEADS // HG           # 8 heads per core
DG = HPG * DK               # 512 projection width per core
N_CORES = B * HG
KT = 128                    # seq tile on the key axis (partitions of S^T)
QC = 512                    # seq column block on the query axis
NKT = S // KT               # 16
NQC = S // QC               # 4
NDT = DG // 128             # 4 planes of Q^T/K^T/O^T
NKD = D // 128              # 8 k-tiles of the model dim
MASK_CONST = 4294967296.0   # +2^32, faithful to the reference
SCALE = 1.0 / np.sqrt(np.float32(DK))   # 1/8
MASKC = np.float32(2.0 ** 29)           # mask * 2^32 * SCALE
NEG_BIAS = -float(2.0 ** 29)

F32 = mybir.dt.float32
F32R = mybir.dt.float32r
BF16 = mybir.dt.bfloat16


def _round_f32r(a: np.ndarray) -> np.ndarray:
    """Round fp32 to the fp32r grid (13-bit mantissa, RNE) like the HW cast."""
    u = np.ascontiguousarray(a, dtype=np.float32).view(np.uint32)
    r = (u + np.uint32(0x1FF) + ((u >> np.uint32(10)) & np.uint32(1))) & np.uint32(0xFFFFFC00)
    return r.view(np.float32)


# ------------------------------------------------------------ classification
def _classify(mask2d: np.ndarray):
    """Block classes per (qc, kt): 0 = all-zero, 1 = all-one, 2 = mixed.
    Also the list of rows with no exactly-1.0 mask entry (host-patched)."""
    m = mask2d
    has1 = (m == 1.0).any(axis=1)
    qfix = np.where(~has1)[0]
    cls = np.empty((NQC, NKT), dtype=np.int8)
    for c in range(NQC):
        sub = m[c * QC:(c + 1) * QC]
        for t in range(NKT):
            blk = sub[:, t * KT:(t + 1) * KT]
            if (blk == 0.0).all():
                cls[c, t] = 0
            elif (blk == 1.0).all():
                cls[c, t] = 1
            else:
                cls[c, t] = 2
    return cls, qfix


# ------------------------------------------------------------- kernel build
def _build(cls: np.ndarray, need_dump: bool):
    comp = [[t for t in range(NKT) if cls[c, t] == 2] for c in range(NQC)]
    ones = [[t for t in range(NKT) if cls[c, t] == 1] for c in range(NQC)]
    n_comp = [len(x) for x in comp]
    n_comp_total = max(1, sum(n_comp))
    comp_off = np.cumsum([0] + n_comp)  # offset of qc's blocks in maskct

    nc = bacc.Bacc("TRN2", target_bir_lowering=False, debug=False,
                   num_devices=N_CORES)

    dt_in = {}
    def din(name, shape, dt):
        dt_in[name] = nc.dram_tensor(name, shape, dt, kind="ExternalInput").ap()
        return dt_in[name]

    xq = din("xq", (128, NKD, S), F32R)        # x_q^T  [p, kt, s]
    xk = din("xk", (128, NKD, S), F32R)
    xv = din("xv", (128, NKD, S), F32R)
    wq = din("wq", (128, NKD, DG), F32R)       # [p, kt, dout]
    wk = din("wk", (128, NKD, DG), F32R)       # pre-scaled by 1/8
    wv = din("wv", (128, NKD, DG), F32R)
    wo = din("wo", (128, NDT, D), F32R)        # [p, plane, dmodel]
    bq = din("bq", (128, NDT), F32)
    bk = din("bk", (128, NDT), F32)            # pre-scaled by 1/8
    bv = din("bv", (1, DG), F32R)
    bo = din("bo", (1, D), F32R)
    maskct = din("maskct", (128, n_comp_total, QC), BF16)  # 2^29 * mask^T blocks

    out = nc.dram_tensor("out", (S, D), F32, kind="ExternalOutput").ap()
    if need_dump:
        kt_out = nc.dram_tensor("kt_out", (128, NDT, S), F32R,
                                kind="ExternalOutput").ap()
        v_out = nc.dram_tensor("v_out", (128, NKT, HPG, DK), F32R,
                               kind="ExternalOutput").ap()

    with tile.TileContext(nc) as tc:
        with (
            tc.tile_pool(name="res", bufs=1) as res,
            tc.tile_pool(name="small", bufs=1) as small,
        ):
            # ---------------- resident tensors
            qt_sb = res.tile([128, NDT, S], F32R, tag="qt")     # Q^T
            kt_sb = res.tile([128, NDT, S], F32R, tag="kt")     # K^T (scaled)
            v_sb = res.tile([128, NKT, HPG, DK + 1], F32R, tag="v")
            wo_sb = res.tile([128, NDT, D], F32R, tag="wo")
            bq_sb = small.tile([128, NDT], F32, tag="bq")
            bk_sb = small.tile([128, NDT], F32, tag="bk")
            bv_sb = small.tile([1, DG], F32R, tag="bv")
            bo_sb = small.tile([1, D], F32R, tag="bo")
            ones_col = small.tile([1, 128], F32R, tag="onesc")
            ones_row = small.tile([1, QC], F32R, tag="onesr")
            ones_k = small.tile([128, 1], F32R, tag="onesk")
            negb = small.tile([128, 1], F32, tag="negb")
            sfx_sb = small.tile([1, NQC, HPG, DK + 1], F32R, tag="sfx")

            nc.vector.memset(ones_col[:].bitcast(F32), 1.0)
            nc.vector.memset(ones_row[:].bitcast(F32), 1.0)
            nc.vector.memset(ones_k[:].bitcast(F32), 1.0)
            nc.vector.memset(negb[:], NEG_BIAS)
            nc.vector.memset(v_sb[:, :, :, DK:DK + 1].bitcast(F32), 1.0)
            nc.sync.dma_start(bq_sb[:], bq[:])
            nc.sync.dma_start(bk_sb[:], bk[:])
            nc.sync.dma_start(bv_sb[:], bv[:])
            nc.sync.dma_start(bo_sb[:], bo[:])

            with (
                tc.tile_pool(name="xstage", bufs=12) as xstage,
                tc.tile_pool(name="wstage", bufs=2) as wstage,
                tc.tile_pool(name="ppsum", bufs=4, space="PSUM") as ppsum,
                tc.tile_pool(name="cpsum", bufs=2, space="PSUM") as cpsum,
            ):
                # ---------------- K^T and Q^T projections
                for which, xin, w_in, b_sb, dst in (
                    ("k", xk, wk, bk_sb, kt_sb),
                    ("q", xq, wq, bq_sb, qt_sb),
                ):
                    wtile = wstage.tile([128, NKD, DG], F32R, tag="w")
                    nc.sync.dma_start(wtile[:], w_in[:])
                    for c in range(NQC):
                        xt = []
                        for k in range(NKD):
                            t = xstage.tile([128, QC], F32R, tag="xt")
                            nc.sync.dma_start(t[:], xin[:, k, c * QC:(c + 1) * QC])
                            xt.append(t)
                        for dtl in range(NDT):
                            ps = ppsum.tile([128, QC], F32, tag="pp")
                            for k in range(NKD):
                                nc.tensor.matmul(
                                    ps[:], wtile[:, k, dtl * 128:(dtl + 1) * 128],
                                    xt[k][:], start=(k == 0), stop=(k == NKD - 1))
                            nc.scalar.activation(
                                dst[:, dtl, c * QC:(c + 1) * QC], ps[:],
                                mybir.ActivationFunctionType.Identity,
                                bias=b_sb[:, dtl:dtl + 1])
                # ---------------- V projection (natural orientation)
                if need_dump:
                    nc.sync.dma_start(kt_out[:], kt_sb[:])
                wvt = wstage.tile([128, NKD, DG], F32R, tag="w")
                nc.sync.dma_start(wvt[:], wv[:])
                for g in range(4):
                    xt = []
                    for k in range(NKD):
                        t = xstage.tile([128, QC], F32R, tag="xt")
                        nc.sync.dma_start(t[:], xv[:, k, g * QC:(g + 1) * QC])
                        xt.append(t)
                    for sl in range(4):
                        st = g * 4 + sl
                        ps = ppsum.tile([128, DG], F32, tag="pp")
                        for k in range(NKD):
                            nc.tensor.matmul(
                                ps[:], xt[k][:, sl * 128:(sl + 1) * 128],
                                wvt[:, k, :], start=(k == 0), stop=False)
                        nc.tensor.matmul(ps[:], ones_col[:, :], bv_sb[:],
                                         start=False, stop=True)
                        for h in range(HPG):
                            nc.vector.tensor_copy(
                                v_sb[:, st, h, 0:DK], ps[:, h * DK:(h + 1) * DK])
                # ------------- column sums of V_ext over each ONES tile set
                for c in range(NQC):
                    if not ones[c]:
                        continue
                    for hh in range(2):
                        cp = cpsum.tile([1, 4, DK + 1], F32, tag="cp")
                        for i, t in enumerate(ones[c]):
                            nc.tensor.matmul(
                                cp[:], ones_k[:],
                                v_sb[:, t, hh * 4:(hh + 1) * 4, :],
                                start=(i == 0), stop=(i == len(ones[c]) - 1))
                        nc.vector.tensor_copy(
                            sfx_sb[:, c, hh * 4:(hh + 1) * 4, :], cp[:])

            # ---------------- attention + output projection
            with (
                tc.tile_pool(name="mstage", bufs=4) as mstage,
                tc.tile_pool(name="ptpool", bufs=2) as ptpool,
                tc.tile_pool(name="otpool", bufs=2) as otpool,
                tc.tile_pool(name="osb", bufs=2) as osb,
                tc.tile_pool(name="outsb", bufs=2) as outsb,
                tc.tile_pool(name="rcpool", bufs=2) as rcpool,
                tc.tile_pool(name="spsum", bufs=2, space="PSUM") as spsum,
                tc.tile_pool(name="opsum", bufs=2, space="PSUM") as opsum,
                tc.tile_pool(name="bpsum", bufs=1, space="PSUM") as bpsum,
                tc.tile_pool(name="wpsum", bufs=1, space="PSUM") as wpsum,
            ):
                mtiles = []
                for c in range(NQC):
                    nct = n_comp[c]
                    if nct:
                        mt = mstage.tile([128, nct, QC], BF16, tag=f"m{nct}")
                        nc.sync.dma_start(
                            mt[:], maskct[:, comp_off[c]:comp_off[c] + nct, :])
                        mtiles.append(mt)
                    else:
                        mtiles.append(None)
                if need_dump:
                    nc.sync.dma_start(v_out[:], v_sb[:, :, :, 0:DK])
                nc.sync.dma_start(wo_sb[:], wo[:])
                for c in range(NQC):
                    nct = n_comp[c]
                    ot_qc = otpool.tile([128, NDT, QC], F32R, tag="ot")
                    mtile = mtiles[c]
                    for h in range(HPG):
                        p0 = 64 * (h % 2)
                        pl = h // 2
                        if nct == 0 and not ones[c]:
                            continue
                        o_ps = opsum.tile([DK + 1, QC], F32, tag="op")
                        first = True
                        if nct:
                            pt = ptpool.tile([128, nct, QC], F32R, tag=f"p{nct}")
                            npair = (nct + 1) // 2
                            for j2 in range(npair):
                                j0 = 2 * j2
                                w = min(2, nct - j0)
                                sp = spsum.tile([128, 2, QC], F32, tag="sp")
                                for jj in range(w):
                                    t = comp[c][j0 + jj]
                                    nc.tensor.matmul(
                                        sp[:, jj, :],
                                        kt_sb[p0:p0 + 64, pl, t * KT:(t + 1) * KT],
                                        qt_sb[p0:p0 + 64, pl, c * QC:(c + 1) * QC],
                                        start=True, stop=True)
                                nc.vector.tensor_tensor(
                                    sp[:, 0:w, :], sp[:, 0:w, :],
                                    mtile[:, j0:j0 + w, :], mybir.AluOpType.add)
                                nc.scalar.activation(
                                    pt[:, j0:j0 + w, :], sp[:, 0:w, :],
                                    mybir.ActivationFunctionType.Exp,
                                    bias=negb[:])
                            for j in range(nct):
                                t = comp[c][j]
                                nc.tensor.matmul(
                                    o_ps[:], v_sb[:, t, h, :], pt[:, j, :],
                                    start=first, stop=False)
                                first = False
                        if ones[c]:
                            nc.tensor.matmul(o_ps[:], sfx_sb[:, c, h, :],
                                             ones_row[:], start=first, stop=True)
                        # normalize: divide by the V_ext ones-column sums
                        s0 = rcpool.tile([1, QC], F32, tag="s0")
                        rc = rcpool.tile([1, QC], F32, tag="rc")
                        rcr = rcpool.tile([1, QC], F32R, tag="rcr")
                        nc.vector.tensor_copy(s0[:], o_ps[DK:DK + 1, :])
                        nc.vector.reciprocal_approx_fast(rc[:], s0[:])
                        nc.vector.tensor_copy(rcr[:], rc[:])
                        b_ps = bpsum.tile([64, QC], F32, tag="bp")
                        nc.tensor.matmul(b_ps[:], ones_col[:, 0:64], rcr[:],
                                         start=True, stop=True)
                        o_sb = osb.tile([64, QC], F32, tag="ob")
                        nc.vector.tensor_copy(o_sb[:], o_ps[0:DK, :])
                        nc.vector.tensor_mul(ot_qc[p0:p0 + 64, pl, :],
                                             o_sb[:], b_ps[:])
                    # ---------- output projection for this q block
                    for qt in range(QC // 128):
                        for ncol in range(D // QC):
                            wp = wpsum.tile([128, QC], F32, tag="wp")
                            for pl in range(NDT):
                                nc.tensor.matmul(
                                    wp[:], ot_qc[:, pl, qt * 128:(qt + 1) * 128],
                                    wo_sb[:, pl, ncol * QC:(ncol + 1) * QC],
                                    start=(pl == 0), stop=False)
                            nc.tensor.matmul(wp[:], ones_col[:, :],
                                             bo_sb[:, ncol * QC:(ncol + 1) * QC],
                                             start=False, stop=True)
                            ob = outsb.tile([128, QC], F32, tag="outb")
                            nc.scalar.copy(ob[:], wp[:])
                            nc.sync.dma_start(
                                out[c * QC + qt * 128:c * QC + (qt + 1) * 128,
                                    ncol * QC:(ncol + 1) * QC], ob[:])

    nc.compile()
    return nc


# ------------------------------------------------------------- host wrapper
_CACHE: dict = {}


def _get_kernel(cls_key, cls, need_dump):
    if cls_key not in _CACHE:
        _CACHE[cls_key] = _build(cls, need_dump)
    return _CACHE[cls_key]


def kernel(queries, keys, values, mask, Wq, bq, Wk, bk, Wv, bv, Wo, bo):
    queries = np.asarray(queries, dtype=np.float32)
    keys = np.asarray(keys, dtype=np.float32)
    values = np.asarray(values, dtype=np.float32)
    mask2d = np.ascontiguousarray(np.asarray(mask, dtype=np.float32).reshape(S, S))
    Wq = np.asarray(Wq, dtype=np.float32); bq_ = np.asarray(bq, dtype=np.float32)
    Wk = np.asarray(Wk, dtype=np.float32); bk_ = np.asarray(bk, dtype=np.float32)
    Wv = np.asarray(Wv, dtype=np.float32); bv_ = np.asarray(bv, dtype=np.float32)
    Wo = np.asarray(Wo, dtype=np.float32); bo_ = np.asarray(bo, dtype=np.float32)

    cls, qfix = _classify(mask2d)
    need_dump = len(qfix) > 0
    comp = [[t for t in range(NKT) if cls[c, t] == 2] for c in range(NQC)]
    n_comp_total = max(1, sum(len(x) for x in comp))
    cls_key = (cls.tobytes(), need_dump)
    nc = _get_kernel(cls_key, cls, need_dump)

    # pack the mixed mask blocks: [p, j, col] = 2^29 * mask[q, k]
    maskct = np.zeros((128, n_comp_total, QC), dtype=np.float32)
    j = 0
    for c in range(NQC):
        for t in comp[c]:
            blk = mask2d[c * QC:(c + 1) * QC, t * KT:(t + 1) * KT]  # [q, k]
            maskct[:, j, :] = blk.T * MASKC
            j += 1
    maskct = maskct.astype(ml_dtypes.bfloat16)

    def prep_xt(x):  # (S, D) -> [128, NKD, S] transposed fp32r
        xt = _round_f32r(x.T)                     # [D, S]
        return np.ascontiguousarray(xt.reshape(NKD, 128, S).transpose(1, 0, 2))

    xts = {}
    in_maps = []
    for core in range(N_CORES):
        b, hg = divmod(core, HG)
        if b not in xts:
            xts[b] = (prep_xt(queries[b]), prep_xt(keys[b]), prep_xt(values[b]))
        sl = slice(hg * DG, (hg + 1) * DG)
        im = {
            "xq": xts[b][0], "xk": xts[b][1], "xv": xts[b][2],
            "wq": np.ascontiguousarray(
                _round_f32r(Wq[:, sl]).reshape(NKD, 128, DG).transpose(1, 0, 2)),
            "wk": np.ascontiguousarray(
                _round_f32r(Wk[:, sl] * np.float32(SCALE)).reshape(NKD, 128, DG).transpose(1, 0, 2)),
            "wv": np.ascontiguousarray(
                _round_f32r(Wv[:, sl]).reshape(NKD, 128, DG).transpose(1, 0, 2)),
            "wo": np.ascontiguousarray(
                _round_f32r(Wo[sl, :]).reshape(NDT, 128, D).transpose(1, 0, 2)),
            "bq": np.ascontiguousarray(bq_[sl].reshape(NDT, 128).T),
            "bk": np.ascontiguousarray((bk_[sl] * np.float32(SCALE)).reshape(NDT, 128).T),
            "bv": _round_f32r(bv_[sl]).reshape(1, DG),
            "bo": (_round_f32r(bo_).reshape(1, D) if hg == 0
                   else np.zeros((1, D), np.float32)),
            "maskct": maskct,
        }
        in_maps.append(im)

    res = bass_utils.run_bass_kernel_spmd(
        nc, in_maps, core_ids=list(range(N_CORES)))

    global LAST_RESULTS, LAST_IN_MAPS
    LAST_RESULTS = res
    LAST_IN_MAPS = in_maps

    out = np.empty((B, S, D), dtype=np.float32)
    for b in range(B):
        out[b] = res.results[b * HG]["out"] + res.results[b * HG + 1]["out"]

    # ---------------- host patch for rows with no masked entry
    if need_dump:
        q = qfix
        mrow = mask2d[q] * np.float32(MASK_CONST)          # [nq, S]
        for b in range(B):
            acc = np.zeros((len(q), D), dtype=np.float32)
            for hg in range(HG):
                r = res.results[b * HG + hg]
                ktd = r["kt_out"]                           # [128, NDT, S]
                K = ktd.transpose(1, 0, 2).reshape(DG, S)   # [dg, S] (scaled)
                vd = r["v_out"]                             # [128, NKT, HPG, DK]
                V = vd.transpose(1, 0, 2, 3).reshape(S, HPG, DK)
                sl = slice(hg * DG, (hg + 1) * DG)
                Qr = queries[b][q] @ Wq[:, sl] + bq_[sl]    # [nq, DG]
                Oc = np.empty((len(q), DG), dtype=np.float32)
                for h in range(HPG):
                    s = (Qr[:, h * DK:(h + 1) * DK] @ K[h * DK:(h + 1) * DK, :])
                    y = s.astype(np.float32) + mrow * np.float32(SCALE)
                    y = y - y.max(axis=1, keepdims=True)
                    e = np.exp(y, dtype=np.float32)
                    p = e / e.sum(axis=1, keepdims=True)
                    Oc[:, h * DK:(h + 1) * DK] = p @ V[:, h, :]
                acc += Oc @ Wo[sl, :]
            out[b][q] = acc + bo_
    return out.reshape(B, S, D)


# revision 19
# speedup vs baseline: 1.0677x; 1.0116x over previous
"""Trainium2 Bass kernel for the 16-head MHA problem (B=4, S=2048, D=1024).

Sharding: 8 cores = 4 batches x 2 head-groups (8 heads each).
Per core: Q/K/V projections (column-split), attention for 8 heads with the
S^T (keys-on-partitions) orientation, row-split output projection; the two
head-group partial outputs per batch are summed on the host.

The reference adds mask*2^32 to the scores BEFORE the 1/sqrt(dk) scale and
softmax.  In fp32 this collapses every masked score to exactly 2^29 (after
the scale) and drives every unmasked score in such rows to exp(-2^29)=0.
We exploit the block structure of the mask (classified at run time from the
actual mask tensor):
  - all-zero mask blocks: contribute nothing -> skipped entirely
  - all-one blocks: P == 1 exactly -> rank-1 (column-sum of V) updates
  - mixed blocks: S matmul + fp32 mask add (reproduces the collapse) + exp
Rows with no masked entry at all (only the last row for the causal-complement
mask) get a true softmax; those are patched on the host from K/V shipped back
from the device.
"""

import numpy as np
import ml_dtypes

import concourse.bass as bass
import concourse.mybir as mybir
import concourse.tile as tile
from concourse import bacc, bass_utils

# ---------------------------------------------------------------- constants
B, S, D = 4, 2048, 1024
HEADS, DK = 16, 64
HG = 2                      # tensor-parallel head groups
HPG = HEADS // HG           # 8 heads per core
DG = HPG * DK               # 512 projection width per core
N_CORES = B * HG
KT = 128                    # seq tile on the key axis (partitions of S^T)
QC = 512                    # seq column block on the query axis
NKT = S // KT               # 16
NQC = S // QC               # 4
NDT = DG // 128             # 4 planes of Q^T/K^T/O^T
NKD = D // 128              # 8 k-tiles of the model dim
MASK_CONST = 4294967296.0   # +2^32, faithful to the reference
SCALE = 1.0 / np.sqrt(np.float32(DK))   # 1/8
MASKC = np.float32(2.0 ** 29)           # mask * 2^32 * SCALE
NEG_BIAS = -float(2.0 ** 29)

F32 = mybir.dt.float32
F32R = mybir.dt.float32r
BF16 = mybir.dt.bfloat16


def _round_f32r(a: np.ndarray) -> np.ndarray:
    """Round fp32 to the fp32r grid (13-bit mantissa, RNE) like the HW cast."""
    u = np.ascontiguousarray(a, dtype=np.float32).view(np.uint32)
    r = (u + np.uint32(0x1FF) + ((u >> np.uint32(10)) & np.uint32(1))) & np.uint32(0xFFFFFC00)
    return r.view(np.float32)


# ------------------------------------------------------------ classification
def _classify(mask2d: np.ndarray):
    """Block classes per (qc, kt): 0 = all-zero, 1 = all-one, 2 = mixed.
    Also the list of rows with no exactly-1.0 mask entry (host-patched)."""
    m = mask2d
    has1 = (m == 1.0).any(axis=1)
    qfix = np.where(~has1)[0]
    cls = np.empty((NQC, NKT), dtype=np.int8)
    for c in range(NQC):
        sub = m[c * QC:(c + 1) * QC]
        for t in range(NKT):
            blk = sub[:, t * KT:(t + 1) * KT]
            if (blk == 0.0).all():
                cls[c, t] = 0
            elif (blk == 1.0).all():
                cls[c, t] = 1
            else:
                cls[c, t] = 2
    return cls, qfix


# ------------------------------------------------------------- kernel build
def _build(cls: np.ndarray, need_dump: bool):
    comp = [[t for t in range(NKT) if cls[c, t] == 2] for c in range(NQC)]
    ones = [[t for t in range(NKT) if cls[c, t] == 1] for c in range(NQC)]
    n_comp = [len(x) for x in comp]
    n_comp_total = max(1, sum(n_comp))
    comp_off = np.cumsum([0] + n_comp)  # offset of qc's blocks in maskct

    nc = bacc.Bacc("TRN2", target_bir_lowering=False, debug=False,
                   num_devices=N_CORES)

    dt_in = {}
    def din(name, shape, dt):
        dt_in[name] = nc.dram_tensor(name, shape, dt, kind="ExternalInput").ap()
        return dt_in[name]

    xq = din("xq", (128, NKD, S), F32R)        # x_q^T  [p, kt, s]
    xk = din("xk", (128, NKD, S), F32R)
    xv = din("xv", (128, NKD, S), F32R)
    wq = din("wq", (128, NKD, DG), F32R)       # [p, kt, dout]
    wk = din("wk", (128, NKD, DG), F32R)       # pre-scaled by 1/8
    wv = din("wv", (128, NKD, DG), F32R)
    wo = din("wo", (128, NDT, D), F32R)        # [p, plane, dmodel]
    bq = din("bq", (128, NDT), F32)
    bk = din("bk", (128, NDT), F32)            # pre-scaled by 1/8
    bv = din("bv", (1, DG), F32R)
    bo = din("bo", (1, D), F32R)
    maskct = din("maskct", (128, n_comp_total, QC), BF16)  # 2^29 * mask^T blocks
    emat = din("emat", (HPG, NDT, 128), F32R)  # head-broadcast indicator

    out = nc.dram_tensor("out", (S, D), F32, kind="ExternalOutput").ap()
    warm_out = nc.dram_tensor("warm_out", (128, QC), F32, kind="ExternalOutput").ap()
    if need_dump:
        kt_out = nc.dram_tensor("kt_out", (128, NDT, S), F32R,
                                kind="ExternalOutput").ap()
        v_out = nc.dram_tensor("v_out", (128, NKT, HPG, DK), F32R,
                               kind="ExternalOutput").ap()

    with tile.TileContext(nc) as tc:
        with (
            tc.tile_pool(name="res", bufs=1) as res,
            tc.tile_pool(name="small", bufs=1) as small,
        ):
            # ---------------- resident tensors
            qt_sb = res.tile([128, NDT, S], F32R, tag="qt")     # Q^T
            kt_sb = res.tile([128, NDT, S], F32R, tag="kt")     # K^T (scaled)
            v_sb = res.tile([128, NKT, HPG, DK + 1], F32R, tag="v")
            wo_sb = res.tile([128, NDT, D], F32R, tag="wo")
            bq_sb = small.tile([128, NDT], F32, tag="bq")
            bk_sb = small.tile([128, NDT], F32, tag="bk")
            bv_sb = small.tile([1, DG], F32R, tag="bv")
            bo_sb = small.tile([1, D], F32R, tag="bo")
            ones_col = small.tile([1, 128], F32R, tag="onesc")
            ones_row = small.tile([1, QC], F32R, tag="onesr")
            ones_k = small.tile([128, 1], F32R, tag="onesk")
            negb = small.tile([128, 1], F32, tag="negb")
            sfx_sb = small.tile([1, NQC, HPG, DK + 1], F32R, tag="sfx")
            e_sb = small.tile([HPG, NDT, 128], F32R, tag="esb")
            scr = small.tile([128, QC], F32R, tag="scr")
            warm_sb = small.tile([128, QC], F32, tag="warm")

            nc.vector.memset(ones_col[:].bitcast(F32), 1.0)
            nc.vector.memset(ones_row[:].bitcast(F32), 1.0)
            nc.vector.memset(ones_k[:].bitcast(F32), 1.0)
            nc.vector.memset(negb[:], NEG_BIAS)
            nc.vector.memset(v_sb[:, :, :, DK:DK + 1].bitcast(F32), 1.0)
            nc.vector.memset(scr[:].bitcast(F32), 1.0)
            nc.sync.dma_start(e_sb[:], emat[:])
            nc.sync.dma_start(bq_sb[:], bq[:])
            nc.sync.dma_start(bk_sb[:], bk[:])
            nc.sync.dma_start(bv_sb[:], bv[:])
            nc.sync.dma_start(bo_sb[:], bo[:])

            with (
                tc.tile_pool(name="xstage", bufs=12) as xstage,
                tc.tile_pool(name="wstage", bufs=18) as wstage,
                tc.tile_pool(name="ppsum", bufs=4, space="PSUM") as ppsum,
                tc.tile_pool(name="cpsum", bufs=2, space="PSUM") as cpsum,
            ):
                # PE warm-up: dense dummy matmuls while the first DMAs land,
                # so HAM reaches K=8/8 before real work and stays there.
                wmp = ppsum.tile([128, QC], F32, tag="pp")
                for r in range(30):
                    nc.tensor.matmul(wmp[:], scr[:, 0:128], scr[:],
                                     start=True, stop=True)
                nc.scalar.copy(warm_sb[:], wmp[:])
                nc.sync.dma_start(warm_out[:], warm_sb[:])

                # ---------------- K^T and Q^T projections
                for which, xin, w_in, b_sb, dst in (
                    ("k", xk, wk, bk_sb, kt_sb),
                    ("q", xq, wq, bq_sb, qt_sb),
                ):
                    wt = []
                    for k in range(NKD):
                        t = wstage.tile([128, DG], F32R, tag="w")
                        nc.sync.dma_start(t[:], w_in[:, k, :])
                        wt.append(t)
                    for c in range(NQC):
                        xt = []
                        for k in range(NKD):
                            t = xstage.tile([128, QC], F32R, tag="xt")
                            nc.sync.dma_start(t[:], xin[:, k, c * QC:(c + 1) * QC])
                            xt.append(t)
                        for dtl in range(NDT):
                            ps = ppsum.tile([128, QC], F32, tag="pp")
                            for k in range(NKD):
                                nc.tensor.matmul(
                                    ps[:], wt[k][:, dtl * 128:(dtl + 1) * 128],
                                    xt[k][:], start=(k == 0), stop=(k == NKD - 1))
                            nc.scalar.activation(
                                dst[:, dtl, c * QC:(c + 1) * QC], ps[:],
                                mybir.ActivationFunctionType.Identity,
                                bias=b_sb[:, dtl:dtl + 1])
                # ---------------- V projection (natural orientation)
                wt = []
                for k in range(NKD):
                    t = wstage.tile([128, DG], F32R, tag="w")
                    nc.sync.dma_start(t[:], wv[:, k, :])
                    wt.append(t)
                for g in range(4):
                    xt = []
                    for k in range(NKD):
                        t = xstage.tile([128, QC], F32R, tag="xt")
                        nc.sync.dma_start(t[:], xv[:, k, g * QC:(g + 1) * QC])
                        xt.append(t)
                    for sl in range(4):
                        st = g * 4 + sl
                        ps = ppsum.tile([128, DG], F32, tag="pp")
                        for k in range(NKD):
                            nc.tensor.matmul(
                                ps[:], xt[k][:, sl * 128:(sl + 1) * 128],
                                wt[k][:], start=(k == 0), stop=False)
                        nc.tensor.matmul(ps[:], ones_col[:, :], bv_sb[:],
                                         start=False, stop=True)
                        for h in range(HPG):
                            nc.vector.tensor_copy(
                                v_sb[:, st, h, 0:DK], ps[:, h * DK:(h + 1) * DK])
                # ------------- column sums of V_ext over each ONES tile set
                for c in range(NQC):
                    if not ones[c]:
                        continue
                    for hh in range(2):
                        cp = cpsum.tile([1, 4, DK + 1], F32, tag="cp")
                        for i, t in enumerate(ones[c]):
                            nc.tensor.matmul(
                                cp[:], ones_k[:],
                                v_sb[:, t, hh * 4:(hh + 1) * 4, :],
                                start=(i == 0), stop=(i == len(ones[c]) - 1))
                        nc.vector.tensor_copy(
                            sfx_sb[:, c, hh * 4:(hh + 1) * 4, :], cp[:])

            # ---------------- attention + output projection
            with (
                tc.tile_pool(name="mstage", bufs=4) as mstage,
                tc.tile_pool(name="ptpool", bufs=2) as ptpool,
                tc.tile_pool(name="otpool", bufs=2) as otpool,
                tc.tile_pool(name="outsb", bufs=2) as outsb,
                tc.tile_pool(name="nrm", bufs=1) as nrm,
                tc.tile_pool(name="spsum", bufs=2, space="PSUM") as spsum,
                tc.tile_pool(name="opsum", bufs=2, space="PSUM") as opsum,
                tc.tile_pool(name="bwpsum", bufs=2, space="PSUM") as bwpsum,
            ):
                mtiles = []
                for c in range(NQC):
                    nct = n_comp[c]
                    if nct:
                        mt = mstage.tile([128, nct, QC], BF16, tag=f"m{nct}")
                        nc.sync.dma_start(
                            mt[:], maskct[:, comp_off[c]:comp_off[c] + nct, :])
                        mtiles.append(mt)
                    else:
                        mtiles.append(None)
                if need_dump:
                    nc.sync.dma_start(kt_out[:], kt_sb[:])
                    nc.sync.dma_start(v_out[:], v_sb[:, :, :, 0:DK])
                nc.sync.dma_start(wo_sb[:], wo[:])
                for c in range(NQC):
                    nct = n_comp[c]
                    ot_qc = otpool.tile([128, NDT, QC], F32R, tag="ot")
                    mtile = mtiles[c]
                    sums8 = nrm.tile([HPG, QC], F32, tag="sums8")
                    rc8 = nrm.tile([HPG, QC], F32, tag="rc8")
                    rcr8 = nrm.tile([HPG, QC], F32R, tag="rcr8")
                    for h in range(HPG):
                        p0 = 64 * (h % 2)
                        pl = h // 2
                        if nct == 0 and not ones[c]:
                            continue
                        o_ps = opsum.tile([DK + 1, QC], F32, tag="op")
                        first = True
                        if nct:
                            pt = ptpool.tile([128, nct, QC], F32R, tag=f"p{nct}")
                            npair = (nct + 1) // 2
                            for j2 in range(npair):
                                j0 = 2 * j2
                                w = min(2, nct - j0)
                                sp = spsum.tile([128, 2, QC], F32, tag="sp")
                                for jj in range(w):
                                    t = comp[c][j0 + jj]
                                    nc.tensor.matmul(
                                        sp[:, jj, :],
                                        kt_sb[p0:p0 + 64, pl, t * KT:(t + 1) * KT],
                                        qt_sb[p0:p0 + 64, pl, c * QC:(c + 1) * QC],
                                        start=True, stop=True)
                                nc.vector.tensor_tensor(
                                    sp[:, 0:w, :], sp[:, 0:w, :],
                                    mtile[:, j0:j0 + w, :], mybir.AluOpType.add)
                                nc.scalar.activation(
                                    pt[:, j0:j0 + w, :], sp[:, 0:w, :],
                                    mybir.ActivationFunctionType.Exp,
                                    bias=negb[:])
                            for j in range(nct):
                                t = comp[c][j]
                                nc.tensor.matmul(
                                    o_ps[:], v_sb[:, t, h, :], pt[:, j, :],
                                    start=first, stop=False)
                                first = False
                        if ones[c]:
                            nc.tensor.matmul(o_ps[:], sfx_sb[:, c, h, :],
                                             ones_row[:], start=first, stop=True)
                        # stash unnormalized head output and its sums row
                        nc.vector.tensor_copy(ot_qc[p0:p0 + 64, pl, :],
                                              o_ps[0:DK, :])
                        sst = nrm.tile([1, QC], F32, tag=f"sst{h % 3}")
                        nc.vector.tensor_copy(sst[:], o_ps[DK:DK + 1, :])
                        nc.sync.dma_start(sums8[h:h + 1, :], sst[:])
                    # ---------- batched normalization for all 8 heads
                    nc.vector.reciprocal_approx_fast(rc8[:], sums8[:])
                    nc.vector.tensor_copy(rcr8[:], rc8[:])
                    for pl in range(NDT):
                        b_ps = bwpsum.tile([128, QC], F32, tag="bw")
                        nc.tensor.matmul(b_ps[:], e_sb[:, pl, :], rcr8[:],
                                         start=True, stop=True)
                        nc.vector.tensor_mul(ot_qc[:, pl, :],
                                             ot_qc[:, pl, :], b_ps[:])
                    # ---------- output projection for this q block
                    for qt in range(QC // 128):
                        for ncol in range(D // QC):
                            wp = bwpsum.tile([128, QC], F32, tag="bw")
                            for pl in range(NDT):
                                nc.tensor.matmul(
                                    wp[:], ot_qc[:, pl, qt * 128:(qt + 1) * 128],
                                    wo_sb[:, pl, ncol * QC:(ncol + 1) * QC],
                                    start=(pl == 0), stop=False)
                            nc.tensor.matmul(wp[:], ones_col[:, :],
                                             bo_sb[:, ncol * QC:(ncol + 1) * QC],
                                             start=False, stop=True)
                            ob = outsb.tile([128, QC], F32, tag="outb")
                            nc.scalar.copy(ob[:], wp[:])
                            nc.sync.dma_start(
                                out[c * QC + qt * 128:c * QC + (qt + 1) * 128,
                                    ncol * QC:(ncol + 1) * QC], ob[:])

    nc.compile()
    return nc


# ------------------------------------------------------------- host wrapper
_CACHE: dict = {}


def _get_kernel(cls_key, cls, need_dump):
    if cls_key not in _CACHE:
        _CACHE[cls_key] = _build(cls, need_dump)
    return _CACHE[cls_key]


def kernel(queries, keys, values, mask, Wq, bq, Wk, bk, Wv, bv, Wo, bo):
    queries = np.asarray(queries, dtype=np.float32)
    keys = np.asarray(keys, dtype=np.float32)
    values = np.asarray(values, dtype=np.float32)
    mask2d = np.ascontiguousarray(np.asarray(mask, dtype=np.float32).reshape(S, S))
    Wq = np.asarray(Wq, dtype=np.float32); bq_ = np.asarray(bq, dtype=np.float32)
    Wk = np.asarray(Wk, dtype=np.float32); bk_ = np.asarray(bk, dtype=np.float32)
    Wv = np.asarray(Wv, dtype=np.float32); bv_ = np.asarray(bv, dtype=np.float32)
    Wo = np.asarray(Wo, dtype=np.float32); bo_ = np.asarray(bo, dtype=np.float32)

    cls, qfix = _classify(mask2d)
    need_dump = len(qfix) > 0
    comp = [[t for t in range(NKT) if cls[c, t] == 2] for c in range(NQC)]
    n_comp_total = max(1, sum(len(x) for x in comp))
    cls_key = (cls.tobytes(), need_dump)
    nc = _get_kernel(cls_key, cls, need_dump)

    # pack the mixed mask blocks: [p, j, col] = 2^29 * mask[q, k]
    maskct = np.zeros((128, n_comp_total, QC), dtype=np.float32)
    j = 0
    for c in range(NQC):
        for t in comp[c]:
            blk = mask2d[c * QC:(c + 1) * QC, t * KT:(t + 1) * KT]  # [q, k]
            maskct[:, j, :] = blk.T * MASKC
            j += 1
    maskct = maskct.astype(ml_dtypes.bfloat16)

    emat_np = np.zeros((HPG, NDT, 128), dtype=np.float32)
    for pl in range(NDT):
        for h2 in range(2):
            emat_np[2 * pl + h2, pl, 64 * h2:64 * h2 + 64] = 1.0

    def prep_xt(x):  # (S, D) -> [128, NKD, S] transposed fp32r
        xt = _round_f32r(x.T)                     # [D, S]
        return np.ascontiguousarray(xt.reshape(NKD, 128, S).transpose(1, 0, 2))

    xts = {}
    in_maps = []
    for core in range(N_CORES):
        b, hg = divmod(core, HG)
        if b not in xts:
            xts[b] = (prep_xt(queries[b]), prep_xt(keys[b]), prep_xt(values[b]))
        sl = slice(hg * DG, (hg + 1) * DG)
        im = {
            "xq": xts[b][0], "xk": xts[b][1], "xv": xts[b][2],
            "wq": np.ascontiguousarray(
                _round_f32r(Wq[:, sl]).reshape(NKD, 128, DG).transpose(1, 0, 2)),
            "wk": np.ascontiguousarray(
                _round_f32r(Wk[:, sl] * np.float32(SCALE)).reshape(NKD, 128, DG).transpose(1, 0, 2)),
            "wv": np.ascontiguousarray(
                _round_f32r(Wv[:, sl]).reshape(NKD, 128, DG).transpose(1, 0, 2)),
            "wo": np.ascontiguousarray(
                _round_f32r(Wo[sl, :]).reshape(NDT, 128, D).transpose(1, 0, 2)),
            "bq": np.ascontiguousarray(bq_[sl].reshape(NDT, 128).T),
            "bk": np.ascontiguousarray((bk_[sl] * np.float32(SCALE)).reshape(NDT, 128).T),
            "bv": _round_f32r(bv_[sl]).reshape(1, DG),
            "bo": (_round_f32r(bo_).reshape(1, D) if hg == 0
                   else np.zeros((1, D), np.float32)),
            "maskct": maskct,
            "emat": emat_np,
        }
        in_maps.append(im)

    res = bass_utils.run_bass_kernel_spmd(
        nc, in_maps, core_ids=list(range(N_CORES)))

    global LAST_RESULTS, LAST_IN_MAPS
    LAST_RESULTS = res
    LAST_IN_MAPS = in_maps

    out = np.empty((B, S, D), dtype=np.float32)
    for b in range(B):
        out[b] = res.results[b * HG]["out"] + res.results[b * HG + 1]["out"]

    # ---------------- host patch for rows with no masked entry
    if need_dump:
        q = qfix
        mrow = mask2d[q] * np.float32(MASK_CONST)          # [nq, S]
        for b in range(B):
            acc = np.zeros((len(q), D), dtype=np.float32)
            for hg in range(HG):
                r = res.results[b * HG + hg]
                ktd = r["kt_out"]                           # [128, NDT, S]
                K = ktd.transpose(1, 0, 2).reshape(DG, S)   # [dg, S] (scaled)
                vd = r["v_out"]                             # [128, NKT, HPG, DK]
                V = vd.transpose(1, 0, 2, 3).reshape(S, HPG, DK)
                sl = slice(hg * DG, (hg + 1) * DG)
                Qr = queries[b][q] @ Wq[:, sl] + bq_[sl]    # [nq, DG]
                Oc = np.empty((len(q), DG), dtype=np.float32)
                for h in range(HPG):
                    s = (Qr[:, h * DK:(h + 1) * DK] @ K[h * DK:(h + 1) * DK, :])
                    y = s.astype(np.float32) + mrow * np.float32(SCALE)
                    y = y - y.max(axis=1, keepdims=True)
                    e = np.exp(y, dtype=np.float32)
                    p = e / e.sum(axis=1, keepdims=True)
                    Oc[:, h * DK:(h + 1) * DK] = p @ V[:, h, :]
                acc += Oc @ Wo[sl, :]
            out[b][q] = acc + bo_
    return out.reshape(B, S, D)
